# revision 10
# baseline (speedup 1.0000x reference)
"""Trainium2 Bass kernel for nn_AttentionBase (8-head attention w/ T5-style
relative-position bias + output projection), sharded head-parallel over 8
NeuronCores.

Per-core program (core c owns head h=c, both batch elements):
  phase 0: build bias lookup Frev[x] = table[bucket(n-1-x), h] on device via a
           one-hot matmul; bounce through DRAM and broadcast-load the shifted
           Toeplitz matrix P[r, c] = Frev[127 + c - r]; expP = exp(SCALE * P).
  phase 1 (per batch b): PE-transpose Q,K tiles into qT,kT ([64, n], head_dim
           on partitions).
  phase 2: for each query block (w=1024) accumulate over key tiles kt:
           S^T[128k, w] = kT_tile^T.T @ qT  (fp32, PSUM)
           araw = exp(SCALE * S^T)          (ACT, PSUM->SBUF, bf16)
           at   = araw * expP_slice         (DVE, bf16 2x mode)
           outT[65, w] += V'[kt]^T.T @ at   (V' has ones column -> row 64 of
                                             outT accumulates softmax denom)
  phase 3: recip denom via exp(-ln(den)) on ACT, broadcast over partitions via
           DRAM bounce, normalize O^T, project per 128-query tile with W_h and
           DMA PSUM->DRAM.
Host: out = sum_c partial_c + b_out.
"""

import math
import sys

sys.path.insert(0, "/opt/trn_rl_repo")

import numpy as np
import ml_dtypes

import concourse.bass as bass
import concourse.bacc as bacc_mod
import concourse.mybir as mybir
import concourse.tile as tile
from concourse.masks import make_identity

NUM_HEADS = 8
HEAD_DIM = 64
MID = 512
OUT_F = 512
NUM_BUCKETS = 32
MAX_DISTANCE = 128
SCALE = HEAD_DIM ** -0.5
N_CORES = 8

F32 = mybir.dt.float32
BF16 = mybir.dt.bfloat16
AF = mybir.ActivationFunctionType


def _bucket_np(rel):
    """Exact numpy port of reference._relative_position_bucket with
    num_buckets=64, max_distance=128 (as the module calls it)."""
    num_buckets = (2 * NUM_BUCKETS) // 2  # 32
    ret = (rel >= 0).astype(np.int32) * num_buckets
    n = np.abs(rel)
    max_exact = max(1, num_buckets // 2)  # 16
    denom = (
        math.log(MAX_DISTANCE / max_exact) if MAX_DISTANCE > max_exact else 1.0
    )
    n_float = np.maximum(n.astype(np.float32), 1.0)
    val_if_large = (
        max_exact + np.log(n_float / max_exact) / denom * (num_buckets - max_exact)
    ).astype(np.int32)
    val_if_large = np.minimum(val_if_large, num_buckets - 1)
    return ret + np.where(n < max_exact, n, val_if_large)


def make_onehot_rev(n):
    """[64, 2n] bf16 one-hot: col x selects table row bucket(n-1-x); the
    matmul table_h^T @ onehot therefore emits Frev[x] = f(n-1-x)."""
    x = np.arange(2 * n - 1, dtype=np.int64)
    delta = (n - 1) - x
    b = _bucket_np(delta)
    oh = np.zeros((2 * NUM_BUCKETS, 2 * n), dtype=ml_dtypes.bfloat16)
    oh[b, x] = 1.0
    return oh


def build_nc(n=4096, w=1024, p_single_dma=False):
    assert n % 128 == 0 and n % w == 0
    mmw = min(512, w)  # matmul free-dim chunk
    Kt = n // 128          # key tiles
    n_qb = n // w          # query blocks
    Wp = 2 * n - 128       # width of the shifted bias matrix P
    Xoh = 2 * n            # one-hot cols (last col zero pad)

    nc = bacc_mod.Bacc()
    q_h = nc.declare_dram_parameter("q_h", [2, n, HEAD_DIM], F32, isOutput=False)
    k_h = nc.declare_dram_parameter("k_h", [2, n, HEAD_DIM], F32, isOutput=False)
    v_h = nc.declare_dram_parameter("v_h", [2, n, HEAD_DIM], F32, isOutput=False)
    table_h = nc.declare_dram_parameter("table_h", [64, 1], BF16, isOutput=False)
    W_h = nc.declare_dram_parameter("W_h", [HEAD_DIM, OUT_F], F32, isOutput=False)
    onehot = nc.declare_dram_parameter("onehot", [64, Xoh], BF16, isOutput=False)
    out_p = nc.declare_dram_parameter("out_partial", [2, n, OUT_F], F32, isOutput=True)

    frev = nc.dram_tensor("frev_scr", (1, 2 * n), F32)
    den_scr = nc.dram_tensor("den_scr", (2, n), F32)

    with tile.TileContext(nc) as tc:
        with (
            tc.tile_pool(name="big", bufs=1) as big,
            tc.tile_pool(name="oacc", bufs=1) as oaccp,
            tc.tile_pool(name="qkT", bufs=2) as qkTp,
            tc.tile_pool(name="vp", bufs=2) as vpp,
            tc.tile_pool(name="stage", bufs=3) as stage,
            tc.tile_pool(name="nat", bufs=4) as natp,
            tc.tile_pool(name="ohp", bufs=2) as ohp,
            tc.tile_pool(name="pqk", bufs=2, space="PSUM") as pqk,
            tc.tile_pool(name="pav", bufs=2, space="PSUM") as pav,
        ):
            # ---- phase 0: constants + bias structure ----
            ident = big.tile([128, 128], F32, tag="ident")
            make_identity(nc, ident)

            tab = big.tile([64, 1], BF16, tag="tab")
            nc.gpsimd.dma_start(tab, table_h[:, :])
            Wt = big.tile([HEAD_DIM, OUT_F], F32, tag="W")
            nc.gpsimd.dma_start(Wt, W_h[:, :])

            # Frev via one-hot matmul, PSUM chunks straight to DRAM scratch
            for ci in range(Xoh // 512):
                oh = ohp.tile([64, 512], BF16, tag="oh")
                nc.gpsimd.dma_start(oh, onehot[:, 512 * ci : 512 * (ci + 1)])
                fp = pqk.tile([1, 512], F32, tag="qk")
                nc.tensor.matmul(fp, tab, oh, start=True, stop=True)
                fsb = ohp.tile([1, 512], F32, tag="fsb")
                nc.vector.tensor_copy(fsb, fp)
                nc.gpsimd.dma_start(frev[:, 512 * ci : 512 * (ci + 1)], fsb)

            tc.strict_bb_all_engine_barrier()
            # P[r, c] = Frev[127 + c - r] -> one broadcast-shift DMA
            P = big.tile([128, Wp], BF16, tag="P")
            if p_single_dma:
                src = bass.AP(tensor=frev, offset=127, ap=[[-1, 128], [1, Wp]])
                nc.gpsimd.dma_start(P, src)
            else:
                for r in range(128):
                    nc.gpsimd.dma_start(P[r : r + 1, :], frev[:, 127 - r : 127 - r + Wp])
            tc.strict_bb_all_engine_barrier()
            expP = big.tile([128, Wp], BF16, tag="expP")
            nc.scalar.activation(expP, P, AF.Exp, scale=SCALE)

            for b in range(2):
                # ---- phase 1: transposes + V' load ----
                qT = qkTp.tile([HEAD_DIM, n], F32, tag="qT")
                kT = qkTp.tile([HEAD_DIM, n], F32, tag="kT")
                qr = q_h[b].rearrange("(t p) d -> t p d", p=128)
                kr = k_h[b].rearrange("(t p) d -> t p d", p=128)
                for t in range(Kt):
                    qn = natp.tile([128, HEAD_DIM], F32, tag="nat")
                    nc.gpsimd.dma_start(qn, qr[t])
                    tp = pqk.tile([HEAD_DIM, 128], F32, tag="qk")
                    nc.tensor.transpose(tp, qn, ident)
                    nc.any.tensor_copy(qT[:, 128 * t : 128 * (t + 1)], tp)
                    kn = natp.tile([128, HEAD_DIM], F32, tag="nat")
                    nc.gpsimd.dma_start(kn, kr[t])
                    tp2 = pqk.tile([HEAD_DIM, 128], F32, tag="qk")
                    nc.tensor.transpose(tp2, kn, ident)
                    nc.any.tensor_copy(kT[:, 128 * t : 128 * (t + 1)], tp2)

                # V' [128, Kt, 65] bf16: col 64 = 1.0 (denominator trick)
                vp = vpp.tile([128, Kt, HEAD_DIM + 1], BF16, tag="vp")
                vr = v_h[b].rearrange("(t p) d -> p t d", p=128)
                nc.gpsimd.dma_start(vp[:, :, 0:HEAD_DIM], vr)  # casting DMA f32->bf16
                nc.vector.memset(vp[:, :, HEAD_DIM : HEAD_DIM + 1], 1.0)

                # ---- phase 2: attention ----
                o65 = oaccp.tile([HEAD_DIM + 1, n], F32, tag="o65")
                for qb in range(n_qb):
                    av = pav.tile([HEAD_DIM + 1, w], F32, tag="av")
                    for kt in range(Kt):
                        st = pqk.tile([128, w], F32, tag="qk")
                        for half in range(w // mmw):
                            nc.tensor.matmul(
                                st[:, mmw * half : mmw * (half + 1)],
                                kT[:, 128 * kt : 128 * (kt + 1)],
                                qT[:, w * qb + mmw * half : w * qb + mmw * (half + 1)],
                                start=True,
                                stop=True,
                            )
                        araw = stage.tile([128, w], BF16, tag="araw")
                        nc.scalar.activation(araw, st, AF.Exp, scale=SCALE)
                        at = stage.tile([128, w], BF16, tag="at")
                        c0 = (n - 128) - 128 * kt + w * qb
                        nc.vector.tensor_mul(at, araw, expP[:, c0 : c0 + w])
                        for half in range(w // mmw):
                            nc.tensor.matmul(
                                av[:, mmw * half : mmw * (half + 1)],
                                vp[:, kt, :],
                                at[:, mmw * half : mmw * (half + 1)],
                                start=(kt == 0),
                                stop=(kt == Kt - 1),
                            )
                    nc.vector.tensor_copy(o65[:, w * qb : w * (qb + 1)], av)

                # ---- phase 3: normalize + project ----
                recb = big.tile([HEAD_DIM, n], F32, tag="recb")
                nc.scalar.activation(recb[0:1, :], o65[HEAD_DIM : HEAD_DIM + 1, :], AF.Ln)
                nc.scalar.activation(recb[32:33, :], recb[0:1, :], AF.Exp, scale=-1.0)
                nc.gpsimd.dma_start(den_scr[b : b + 1, :], recb[32:33, :])
                bsrc = bass.AP(
                    tensor=den_scr, offset=b * n, ap=[[0, HEAD_DIM], [1, n]]
                )
                nc.gpsimd.dma_start(recb, bsrc)
                on = big.tile([HEAD_DIM, n], F32, tag="on")
                nc.vector.tensor_mul(on, o65[0:HEAD_DIM, :], recb)

                for qt in range(Kt):
                    pj = pav.tile([128, OUT_F], F32, tag="av")
                    nc.tensor.matmul(
                        pj,
                        on[:, 128 * qt : 128 * (qt + 1)],
                        Wt,
                        start=True,
                        stop=True,
                    )
                    osb = stage.tile([128, OUT_F], F32, tag="osb")
                    nc.any.tensor_copy(osb, pj)
                    nc.gpsimd.dma_start(out_p[b, 128 * qt : 128 * (qt + 1), :], osb)

    nc.compile()
    return nc


def make_in_maps(q, k, v, rel_bias_table, W_out, n):
    """Shard full inputs per core (core c <-> head c)."""
    oh = make_onehot_rev(n)
    in_maps = []
    for c in range(N_CORES):
        sl = slice(HEAD_DIM * c, HEAD_DIM * (c + 1))
        in_maps.append(
            {
                "q_h": np.ascontiguousarray(q[:, :, sl]),
                "k_h": np.ascontiguousarray(k[:, :, sl]),
                "v_h": np.ascontiguousarray(v[:, :, sl]),
                "table_h": np.ascontiguousarray(
                    rel_bias_table[:, c : c + 1]
                ).astype(ml_dtypes.bfloat16),
                "W_h": np.ascontiguousarray(W_out[sl, :]),
                "onehot": oh,
            }
        )
    return in_maps


_NC_CACHE = {}


def _get_nc(n, w):
    key = (n, w)
    if key not in _NC_CACHE:
        _NC_CACHE[key] = build_nc(n=n, w=w)
    return _NC_CACHE[key]


def kernel(q, k, v, rel_bias_table, W_out, b_out):
    from concourse.bass_utils import run_bass_kernel_spmd

    q = np.asarray(q, dtype=np.float32)
    k = np.asarray(k, dtype=np.float32)
    v = np.asarray(v, dtype=np.float32)
    rel_bias_table = np.asarray(rel_bias_table, dtype=np.float32)
    W_out = np.asarray(W_out, dtype=np.float32)
    b_out = np.asarray(b_out, dtype=np.float32)

    n = q.shape[1]
    w = min(1024, n)
    nc = _get_nc(n, w)
    in_maps = make_in_maps(q, k, v, rel_bias_table, W_out, n)
    res = run_bass_kernel_spmd(nc, in_maps, core_ids=list(range(N_CORES)))
    acc = np.zeros((2, n, OUT_F), dtype=np.float64)
    for r in res.results:
        acc += r["out_partial"].astype(np.float64)
    return (acc + b_out.astype(np.float64)).astype(np.float32)


# revision 11
# speedup vs baseline: 66.3393x; 66.3393x over previous
"""Trainium2 Bass kernel for nn_AttentionBase (8-head attention w/ T5-style
relative-position bias + output projection), sharded head-parallel over 8
NeuronCores.

Per-core program (core c owns head h=c, both batch elements):
  phase 0: build bias lookup Frev[x] = table[bucket(n-1-x), h] on device via a
           one-hot matmul; bounce through DRAM and broadcast-load the shifted
           Toeplitz matrix P[r, c] = Frev[127 + c - r]; expP = exp(SCALE * P).
  phase 1 (per batch b): PE-transpose Q,K tiles into qT,kT ([64, n], head_dim
           on partitions).
  phase 2: for each query block (w=1024) accumulate over key tiles kt:
           S^T[128k, w] = kT_tile^T.T @ qT  (fp32, PSUM)
           araw = exp(SCALE * S^T)          (ACT, PSUM->SBUF, bf16)
           at   = araw * expP_slice         (DVE, bf16 2x mode)
           outT[65, w] += V'[kt]^T.T @ at   (V' has ones column -> row 64 of
                                             outT accumulates softmax denom)
  phase 3: recip denom via exp(-ln(den)) on ACT, broadcast over partitions via
           DRAM bounce, normalize O^T, project per 128-query tile with W_h and
           DMA PSUM->DRAM.
Host: out = sum_c partial_c + b_out.
"""

import math
import sys

sys.path.insert(0, "/opt/trn_rl_repo")

import numpy as np
import ml_dtypes

import concourse.bass as bass
import concourse.bacc as bacc_mod
import concourse.mybir as mybir
import concourse.tile as tile
from concourse.masks import make_identity

NUM_HEADS = 8
HEAD_DIM = 64
MID = 512
OUT_F = 512
NUM_BUCKETS = 32
MAX_DISTANCE = 128
SCALE = HEAD_DIM ** -0.5
N_CORES = 8

F32 = mybir.dt.float32
BF16 = mybir.dt.bfloat16
AF = mybir.ActivationFunctionType


def _bucket_np(rel):
    """Exact numpy port of reference._relative_position_bucket with
    num_buckets=64, max_distance=128 (as the module calls it)."""
    num_buckets = (2 * NUM_BUCKETS) // 2  # 32
    ret = (rel >= 0).astype(np.int32) * num_buckets
    n = np.abs(rel)
    max_exact = max(1, num_buckets // 2)  # 16
    denom = (
        math.log(MAX_DISTANCE / max_exact) if MAX_DISTANCE > max_exact else 1.0
    )
    n_float = np.maximum(n.astype(np.float32), 1.0)
    val_if_large = (
        max_exact + np.log(n_float / max_exact) / denom * (num_buckets - max_exact)
    ).astype(np.int32)
    val_if_large = np.minimum(val_if_large, num_buckets - 1)
    return ret + np.where(n < max_exact, n, val_if_large)


def make_onehot_rev(n):
    """[64, 2n] bf16 one-hot: col x selects table row bucket(n-1-x); the
    matmul table_h^T @ onehot therefore emits Frev[x] = f(n-1-x)."""
    x = np.arange(2 * n - 1, dtype=np.int64)
    delta = (n - 1) - x
    b = _bucket_np(delta)
    oh = np.zeros((2 * NUM_BUCKETS, 2 * n), dtype=ml_dtypes.bfloat16)
    oh[b, x] = 1.0
    return oh


def build_nc(n=4096, w=1024, p_single_dma=False, reps=1):
    assert n % 128 == 0 and n % w == 0
    mmw = min(512, w)  # matmul free-dim chunk
    Kt = n // 128          # key tiles
    n_qb = n // w          # query blocks
    Wp = 2 * n - 128       # width of the shifted bias matrix P
    Xoh = 2 * n            # one-hot cols (last col zero pad)

    nc = bacc_mod.Bacc()
    q_h = nc.declare_dram_parameter("q_h", [2, n, HEAD_DIM], F32, isOutput=False)
    k_h = nc.declare_dram_parameter("k_h", [2, n, HEAD_DIM], F32, isOutput=False)
    v_h = nc.declare_dram_parameter("v_h", [2, n, HEAD_DIM], F32, isOutput=False)
    table_h = nc.declare_dram_parameter("table_h", [64, 1], BF16, isOutput=False)
    W_h = nc.declare_dram_parameter("W_h", [HEAD_DIM, OUT_F], F32, isOutput=False)
    onehot = nc.declare_dram_parameter("onehot", [64, Xoh], BF16, isOutput=False)
    out_p = nc.declare_dram_parameter("out_partial", [2, n, OUT_F], F32, isOutput=True)

    frev = nc.dram_tensor("frev_scr", (1, 2 * n), F32)
    den_scr = nc.dram_tensor("den_scr", (2, n), F32)

    with tile.TileContext(nc) as tc:
        with (
            tc.tile_pool(name="big", bufs=1) as big,
            tc.tile_pool(name="oacc", bufs=1) as oaccp,
            tc.tile_pool(name="qkT", bufs=2) as qkTp,
            tc.tile_pool(name="vp", bufs=2) as vpp,
            tc.tile_pool(name="stage", bufs=3) as stage,
            tc.tile_pool(name="nat", bufs=4) as natp,
            tc.tile_pool(name="ohp", bufs=2) as ohp,
            tc.tile_pool(name="pqk", bufs=2, space="PSUM") as pqk,
            tc.tile_pool(name="pav", bufs=2, space="PSUM") as pav,
        ):
            # ---- phase 0: constants + bias structure ----
            ident = big.tile([128, 128], F32, tag="ident")
            make_identity(nc, ident)

            tab = big.tile([64, 1], BF16, tag="tab")
            nc.gpsimd.dma_start(tab, table_h[:, :])
            Wt = big.tile([HEAD_DIM, OUT_F], F32, tag="W")
            nc.gpsimd.dma_start(Wt, W_h[:, :])

            # Frev via one-hot matmul, PSUM chunks straight to DRAM scratch
            for ci in range(Xoh // 512):
                oh = ohp.tile([64, 512], BF16, tag="oh")
                nc.gpsimd.dma_start(oh, onehot[:, 512 * ci : 512 * (ci + 1)])
                fp = pqk.tile([1, 512], F32, tag="qk")
                nc.tensor.matmul(fp, tab, oh, start=True, stop=True)
                fsb = ohp.tile([1, 512], F32, tag="fsb")
                nc.vector.tensor_copy(fsb, fp)
                nc.gpsimd.dma_start(frev[:, 512 * ci : 512 * (ci + 1)], fsb)

            tc.strict_bb_all_engine_barrier()
            # P[r, c] = Frev[127 + c - r] -> one broadcast-shift DMA
            P = big.tile([128, Wp], BF16, tag="P")
            if p_single_dma:
                src = bass.AP(tensor=frev, offset=127, ap=[[-1, 128], [1, Wp]])
                nc.gpsimd.dma_start(P, src)
            else:
                for r in range(128):
                    nc.gpsimd.dma_start(P[r : r + 1, :], frev[:, 127 - r : 127 - r + Wp])
            tc.strict_bb_all_engine_barrier()
            expP = big.tile([128, Wp], BF16, tag="expP")
            nc.scalar.activation(expP, P, AF.Exp, scale=SCALE)

            for rep in range(reps):
             for b in range(2):
                # ---- phase 1: transposes + V' load ----
                qT = qkTp.tile([HEAD_DIM, n], F32, tag="qT")
                kT = qkTp.tile([HEAD_DIM, n], F32, tag="kT")
                qr = q_h[b].rearrange("(t p) d -> t p d", p=128)
                kr = k_h[b].rearrange("(t p) d -> t p d", p=128)
                for t in range(Kt):
                    qn = natp.tile([128, HEAD_DIM], F32, tag="nat")
                    nc.gpsimd.dma_start(qn, qr[t])
                    tp = pqk.tile([HEAD_DIM, 128], F32, tag="qk")
                    nc.tensor.transpose(tp, qn, ident)
                    nc.any.tensor_copy(qT[:, 128 * t : 128 * (t + 1)], tp)
                    kn = natp.tile([128, HEAD_DIM], F32, tag="nat")
                    nc.gpsimd.dma_start(kn, kr[t])
                    tp2 = pqk.tile([HEAD_DIM, 128], F32, tag="qk")
                    nc.tensor.transpose(tp2, kn, ident)
                    nc.any.tensor_copy(kT[:, 128 * t : 128 * (t + 1)], tp2)

                # V' [128, Kt, 65] bf16: col 64 = 1.0 (denominator trick)
                vp = vpp.tile([128, Kt, HEAD_DIM + 1], BF16, tag="vp")
                vr = v_h[b].rearrange("(t p) d -> p t d", p=128)
                nc.gpsimd.dma_start(vp[:, :, 0:HEAD_DIM], vr)  # casting DMA f32->bf16
                nc.vector.memset(vp[:, :, HEAD_DIM : HEAD_DIM + 1], 1.0)

                # ---- phase 2: attention ----
                o65 = oaccp.tile([HEAD_DIM + 1, n], F32, tag="o65")
                for qb in range(n_qb):
                    av = pav.tile([HEAD_DIM + 1, w], F32, tag="av")
                    for kt in range(Kt):
                        st = pqk.tile([128, w], F32, tag="qk")
                        for half in range(w // mmw):
                            nc.tensor.matmul(
                                st[:, mmw * half : mmw * (half + 1)],
                                kT[:, 128 * kt : 128 * (kt + 1)],
                                qT[:, w * qb + mmw * half : w * qb + mmw * (half + 1)],
                                start=True,
                                stop=True,
                            )
                        araw = stage.tile([128, w], BF16, tag="araw")
                        nc.scalar.activation(araw, st, AF.Exp, scale=SCALE)
                        at = stage.tile([128, w], BF16, tag="at")
                        c0 = (n - 128) - 128 * kt + w * qb
                        nc.vector.tensor_mul(at, araw, expP[:, c0 : c0 + w])
                        for half in range(w // mmw):
                            nc.tensor.matmul(
                                av[:, mmw * half : mmw * (half + 1)],
                                vp[:, kt, :],
                                at[:, mmw * half : mmw * (half + 1)],
                                start=(kt == 0),
                                stop=(kt == Kt - 1),
                            )
                    nc.vector.tensor_copy(o65[:, w * qb : w * (qb + 1)], av)

                # ---- phase 3: normalize + project ----
                recb = big.tile([HEAD_DIM, n], F32, tag="recb")
                nc.scalar.activation(recb[0:1, :], o65[HEAD_DIM : HEAD_DIM + 1, :], AF.Ln)
                nc.scalar.activation(recb[32:33, :], recb[0:1, :], AF.Exp, scale=-1.0)
                nc.gpsimd.dma_start(den_scr[b : b + 1, :], recb[32:33, :])
                bsrc = bass.AP(
                    tensor=den_scr, offset=b * n, ap=[[0, HEAD_DIM], [1, n]]
                )
                nc.gpsimd.dma_start(recb, bsrc)
                on = big.tile([HEAD_DIM, n], F32, tag="on")
                nc.vector.tensor_mul(on, o65[0:HEAD_DIM, :], recb)

                for qt in range(Kt):
                    pj = pav.tile([128, OUT_F], F32, tag="av")
                    nc.tensor.matmul(
                        pj,
                        on[:, 128 * qt : 128 * (qt + 1)],
                        Wt,
                        start=True,
                        stop=True,
                    )
                    osb = stage.tile([128, OUT_F], F32, tag="osb")
                    nc.any.tensor_copy(osb, pj)
                    nc.gpsimd.dma_start(out_p[b, 128 * qt : 128 * (qt + 1), :], osb)

    nc.compile()
    return nc


def make_in_maps(q, k, v, rel_bias_table, W_out, n):
    """Shard full inputs per core (core c <-> head c)."""
    oh = make_onehot_rev(n)
    in_maps = []
    for c in range(N_CORES):
        sl = slice(HEAD_DIM * c, HEAD_DIM * (c + 1))
        in_maps.append(
            {
                "q_h": np.ascontiguousarray(q[:, :, sl]),
                "k_h": np.ascontiguousarray(k[:, :, sl]),
                "v_h": np.ascontiguousarray(v[:, :, sl]),
                "table_h": np.ascontiguousarray(
                    rel_bias_table[:, c : c + 1]
                ).astype(ml_dtypes.bfloat16),
                "W_h": np.ascontiguousarray(W_out[sl, :]),
                "onehot": oh,
            }
        )
    return in_maps


_NC_CACHE = {}


def _get_nc(n, w):
    key = (n, w)
    if key not in _NC_CACHE:
        _NC_CACHE[key] = build_nc(n=n, w=w)
    return _NC_CACHE[key]


def kernel(q, k, v, rel_bias_table, W_out, b_out):
    from concourse.bass_utils import run_bass_kernel_spmd

    q = np.asarray(q, dtype=np.float32)
    k = np.asarray(k, dtype=np.float32)
    v = np.asarray(v, dtype=np.float32)
    rel_bias_table = np.asarray(rel_bias_table, dtype=np.float32)
    W_out = np.asarray(W_out, dtype=np.float32)
    b_out = np.asarray(b_out, dtype=np.float32)

    n = q.shape[1]
    w = min(1024, n)
    nc = _get_nc(n, w)
    in_maps = make_in_maps(q, k, v, rel_bias_table, W_out, n)
    res = run_bass_kernel_spmd(nc, in_maps, core_ids=list(range(N_CORES)))
    acc = np.zeros((2, n, OUT_F), dtype=np.float64)
    for r in res.results:
        acc += r["out_partial"].astype(np.float64)
    return (acc + b_out.astype(np.float64)).astype(np.float32)
